# revision 6
# baseline (speedup 1.0000x reference)
"""Trainium2 Bass kernel: 2-layer Chebyshev graph conv (K=5) + 3-layer MLP head.

Distribution over 8 NeuronCores:
  - The adjacency a [8192, 8192] is row-sharded; core i owns graph rows
    [1024*i, 1024*(i+1)), passed host-transposed (aT: [8192, 1024]) so the
    contraction dim m is the partition dim of the stationary matmul operand.
  - Chebyshev states T_k live in node-major [N, B*F] layout.  Each
    application of `a` streams the aT shard and the (all-gathered) full T
    from HBM, accumulating psum[n_chunk] over 64 m-chunks; the new local
    1024 rows are then AllGathered for the next application.
  - conv outputs accumulate locally in [n, (b, f)] layout via
    PE-transposes + block-diagonal weight matmuls.
  - fc1 is contraction(row)-sharded over nodes which exactly matches the
    conv2 output shard (flat index = n*F2+f2); partial [16, 512] results
    are AllReduced, then fc2/fc3/softmax run redundantly on every core.
  - Matmuls run as float32r (fp32 bits on the fast PE path); the graph
    stream can be switched to bf16 via KER_DT_GRAPH=bf16.
"""

import os
import sys

import numpy as np

for _p in ("/opt/trn_rl_repo", "/root/.axon_site/_ro/trn_rl_repo"):
    if os.path.isdir(_p) and _p not in sys.path:
        sys.path.insert(0, _p)

P = 128          # SBUF partitions
N = 8192         # nodes
B = 16           # batch
F_IN = 2
F1 = 32
F2 = 32
K = 5            # Chebyshev order
NCORES = 8
R = N // NCORES  # nodes per core (1024)
MC = N // P      # m-chunks (64)
NJ = R // P      # local n-chunks (8)
C1 = 64          # padded conv1 state width (real = B*F_IN = 32)
C2 = B * F1      # 512
M1, M2, M3 = 512, 128, 2

DT_GRAPH = os.environ.get("KER_DT_GRAPH", "f32")   # "f32" (f32r matmul) | "bf16"
DT_FC1 = os.environ.get("KER_DT_FC1", "f32")       # "f32" | "bf16"

_CACHE = {}


def build_kernel():
    from concourse import bacc, mybir, tile
    from concourse.masks import make_identity

    dt = mybir.dt
    f32 = dt.float32
    f32r = dt.float32r
    g_dt = f32r if DT_GRAPH == "f32" else dt.bfloat16
    fc1_dt = f32r if DT_FC1 == "f32" else dt.bfloat16
    Alu = mybir.AluOpType
    Act = mybir.ActivationFunctionType
    RG = [list(range(NCORES))]

    nc = bacc.Bacc(
        "TRN2",
        target_bir_lowering=False,
        debug=False,
        enable_asserts=False,
        num_devices=NCORES,
    )

    # ------------------------- DRAM I/O -------------------------
    at_d = nc.dram_tensor("at", [N, R], g_dt, kind="ExternalInput").ap()
    x2dp_d = nc.dram_tensor("x2dp", [N, C1], g_dt, kind="ExternalInput").ap()
    xloc_d = nc.dram_tensor("xloc", [P, NJ * C1], f32, kind="ExternalInput").ap()
    w1bd_d = nc.dram_tensor("w1bd", [K, C1, C2], f32r, kind="ExternalInput").ap()
    w2bd_d = nc.dram_tensor("w2bd", [K, P, P], f32r, kind="ExternalInput").ap()
    fw1s_d = nc.dram_tensor("fw1s", [R * F2, M1], fc1_dt, kind="ExternalInput").ap()
    fw2_d = nc.dram_tensor("fw2", [M1, M2], f32r, kind="ExternalInput").ap()
    fw3_d = nc.dram_tensor("fw3", [M2, M3], f32r, kind="ExternalInput").ap()
    b1r_d = nc.dram_tensor("b1r", [P, C2], f32, kind="ExternalInput").ap()
    b2r_d = nc.dram_tensor("b2r", [P, C2], f32, kind="ExternalInput").ap()
    fb1r_d = nc.dram_tensor("fb1r", [B, M1], f32, kind="ExternalInput").ap()
    fb2r_d = nc.dram_tensor("fb2r", [B, M2], f32, kind="ExternalInput").ap()
    fb3r_d = nc.dram_tensor("fb3r", [B, M3], f32, kind="ExternalInput").ap()
    out_d = nc.dram_tensor("out", [B, M3], f32, kind="ExternalOutput").ap()

    def cc_spill(out_ap, in_ap):
        """DMA an f32 SBUF tile into a g_dt DRAM cc buffer."""
        if DT_GRAPH == "f32":
            nc.sync.dma_start(out=out_ap, in_=in_ap.bitcast(f32r))
        else:
            nc.gpsimd.dma_start(out=out_ap, in_=in_ap)  # SWDGE casts f32->bf16

    with tile.TileContext(nc) as tc:
        with (
            tc.tile_pool(name="consts", bufs=1) as consts,
            tc.tile_pool(name="tf1p", bufs=1) as tf1p,
            tc.tile_pool(name="tf2p", bufs=12) as tf2p,
            tc.tile_pool(name="atp", bufs=6) as atp,
            tc.tile_pool(name="loc1p", bufs=2) as loc1p,
            tc.tile_pool(name="locp", bufs=16) as locp,
            tc.tile_pool(name="out2p", bufs=8) as out2p,
            tc.tile_pool(name="ttp", bufs=4) as ttp,
            tc.tile_pool(name="fwp", bufs=6) as fwp,
            tc.tile_pool(name="fcp", bufs=2) as fcp,
            tc.tile_pool(name="psum", bufs=8, space="PSUM") as psp,
            tc.tile_pool(name="dram", bufs=2, space="DRAM") as drp,
        ):
            # ------------------------- constants -------------------------
            ident = consts.tile([P, P], f32)
            make_identity(nc, ident)
            w1bd = consts.tile([C1, K * C2], f32r)
            for k in range(K):
                nc.sync.dma_start(out=w1bd[:, k * C2:(k + 1) * C2], in_=w1bd_d[k])
            w2bd = consts.tile([P, K * P], f32r)
            for k in range(K):
                nc.sync.dma_start(out=w2bd[:, k * P:(k + 1) * P], in_=w2bd_d[k])
            b1r = consts.tile([P, C2], f32)
            nc.sync.dma_start(out=b1r[:], in_=b1r_d[:])
            b2r = consts.tile([P, C2], f32)
            nc.sync.dma_start(out=b2r[:], in_=b2r_d[:])
            fb1r = consts.tile([B, M1], f32)
            nc.sync.dma_start(out=fb1r[:], in_=fb1r_d[:])
            fb2r = consts.tile([B, M2], f32)
            nc.sync.dma_start(out=fb2r[:], in_=fb2r_d[:])
            fb3r = consts.tile([B, M3], f32)
            nc.sync.dma_start(out=fb3r[:], in_=fb3r_d[:])
            fw3sb = consts.tile([M2, M3], f32r)
            nc.sync.dma_start(out=fw3sb[:], in_=fw3_d[:])

            # out1 accumulators; later double as conv2's h-local/T-even set
            out1 = []
            for j in range(NJ):
                t = locp.tile([P, C2], f32, tag="loc", name=f"out1_{j}")
                nc.vector.memset(t[:], 0.0)
                out1.append(t)

            # =============================================================
            # conv1 (padded state width C1; psum garbage in the pad columns
            # is harmless: w1bd pad rows are zero)
            # =============================================================
            def conv1_feature(tk_loc, k):
                """out1[j] += transpose(T_k[j]).T @ w1bd[k]"""
                for j in range(NJ):
                    tt_ps = psp.tile([C1, P], f32, tag="ps", name=f"c1ttps_{k}_{j}")
                    nc.tensor.transpose(
                        tt_ps[:], tk_loc[:, j * C1:(j + 1) * C1], ident[:]
                    )
                    tt = ttp.tile([C1, P], f32r, tag="tt", name=f"c1tt_{k}_{j}")
                    nc.vector.tensor_copy(tt[:], tt_ps[:])
                    f_ps = psp.tile([P, C2], f32, tag="ps", name=f"c1fps_{k}_{j}")
                    nc.tensor.matmul(
                        f_ps[:],
                        tt[:],
                        w1bd[:, k * C2:(k + 1) * C2],
                        start=True,
                        stop=True,
                    )
                    nc.vector.tensor_add(out1[j][:], out1[j][:], f_ps[:])

            # T0 = x: full copy (graph rhs) + local rows
            tf1 = tf1p.tile([P, MC * C1], g_dt, tag="tf1", name="tf1_t0")
            nc.sync.dma_start(
                out=tf1.rearrange("p (mc f) -> p mc f", f=C1),
                in_=x2dp_d.rearrange("(mc p) f -> p mc f", p=P),
            )
            locA1 = loc1p.tile([P, NJ * C1], f32, tag="loc1", name="locA1")
            nc.sync.dma_start(out=locA1[:], in_=xloc_d[:])
            locB1 = loc1p.tile([P, NJ * C1], f32, tag="loc1", name="locB1")
            conv1_feature(locA1, 0)

            for k in range(1, K):
                ps_g = [
                    psp.tile([P, C1], f32, tag="ps", name=f"c1g_{k}_{j}")
                    for j in range(NJ)
                ]
                for mc in range(MC):
                    att = atp.tile([P, R], g_dt, tag="at", name=f"c1at_{k}_{mc}")
                    nc.sync.dma_start(out=att[:], in_=at_d[mc * P:(mc + 1) * P, :])
                    rhs = tf1[:, mc * C1:(mc + 1) * C1]
                    for j in range(NJ):
                        nc.tensor.matmul(
                            ps_g[j][:],
                            att[:, j * P:(j + 1) * P],
                            rhs,
                            start=(mc == 0),
                            stop=(mc == MC - 1),
                        )
                # combine: T_k = 2*(a @ T_{k-1}) - T_{k-2}   (T_1 = a @ T_0)
                dst = locB1 if k % 2 == 1 else locA1
                for j in range(NJ):
                    sl = dst[:, j * C1:(j + 1) * C1]
                    if k == 1:
                        nc.vector.tensor_copy(sl, ps_g[j][:])
                    else:
                        nc.vector.scalar_tensor_tensor(
                            sl, ps_g[j][:], 2.0, sl, Alu.mult, Alu.subtract
                        )
                conv1_feature(dst, k)
                if k < K - 1:
                    cc_in = drp.tile([R, C1], g_dt, tag="cc1i", name=f"cc1i_{k}")
                    cc_spill(
                        cc_in.rearrange("(j p) f -> p j f", p=P),
                        dst.rearrange("p (j f) -> p j f", f=C1),
                    )
                    cc_out = drp.tile(
                        [N, C1], g_dt, tag="cc1o", addr_space="Shared",
                        name=f"cc1o_{k}",
                    )
                    nc.gpsimd.collective_compute(
                        "AllGather",
                        Alu.bypass,
                        replica_groups=RG,
                        ins=[cc_in.opt()],
                        outs=[cc_out.opt()],
                    )
                    tf1 = tf1p.tile([P, MC * C1], g_dt, tag="tf1", name=f"tf1_{k}")
                    nc.sync.dma_start(
                        out=tf1.rearrange("p (mc f) -> p mc f", f=C1),
                        in_=cc_out.rearrange("(mc p) f -> p mc f", p=P),
                    )

            # conv1 epilogue: h = relu(out1 + b1); gather h
            for j in range(NJ):
                nc.vector.tensor_add(out1[j][:], out1[j][:], b1r[:])
                nc.vector.tensor_relu(out1[j][:], out1[j][:])
            cc_hi = drp.tile([R, C2], g_dt, tag="cc2i", name="cc_hi")
            for j in range(NJ):
                cc_spill(cc_hi[j * P:(j + 1) * P, :], out1[j][:])
            cc_h = drp.tile(
                [N, C2], g_dt, tag="cc2o", addr_space="Shared", name="cc_h"
            )
            nc.gpsimd.collective_compute(
                "AllGather", Alu.bypass, replica_groups=RG,
                ins=[cc_hi.opt()], outs=[cc_h.opt()],
            )

            # =============================================================
            # conv2 (state width C2 = 512)
            # =============================================================
            out2 = []
            for j in range(NJ):
                t = out2p.tile([P, C2], f32, tag="out2", name=f"out2_{j}")
                nc.vector.memset(t[:], 0.0)
                out2.append(t)

            def conv2_feature(tk_set, k):
                """out2[j] += sum_c transpose(T_k[j] colblock c).T @ w2bd[k]"""
                for j in range(NJ):
                    f_ps = psp.tile([P, C2], f32, tag="ps", name=f"c2fps_{k}_{j}")
                    for c in range(4):
                        tt_ps = psp.tile(
                            [P, P], f32, tag="ps", name=f"c2ttps_{k}_{j}_{c}"
                        )
                        nc.tensor.transpose(
                            tt_ps[:], tk_set[j][:, c * P:(c + 1) * P], ident[:]
                        )
                        tt = ttp.tile([P, P], f32r, tag="tt", name=f"c2tt_{k}_{j}_{c}")
                        nc.vector.tensor_copy(tt[:], tt_ps[:])
                        nc.tensor.matmul(
                            f_ps[:, c * P:(c + 1) * P],
                            tt[:],
                            w2bd[:, k * P:(k + 1) * P],
                            start=True,
                            stop=True,
                        )
                    nc.vector.tensor_add(out2[j][:], out2[j][:], f_ps[:])

            locB = [
                locp.tile([P, C2], f32, tag="loc", name=f"locB_{j}")
                for j in range(NJ)
            ]
            conv2_feature(out1, 0)  # T0 = h (local rows live in out1 tiles)

            src = cc_h
            for k in range(1, K):
                ps_g = [
                    psp.tile([P, C2], f32, tag="ps", name=f"c2g_{k}_{j}")
                    for j in range(NJ)
                ]
                for mc in range(MC):
                    att = atp.tile([P, R], g_dt, tag="at", name=f"c2at_{k}_{mc}")
                    nc.sync.dma_start(out=att[:], in_=at_d[mc * P:(mc + 1) * P, :])
                    tft = tf2p.tile([P, C2], g_dt, tag="tf2", name=f"tf2_{k}_{mc}")
                    nc.sync.dma_start(out=tft[:], in_=src[mc * P:(mc + 1) * P, :])
                    for j in range(NJ):
                        nc.tensor.matmul(
                            ps_g[j][:],
                            att[:, j * P:(j + 1) * P],
                            tft[:],
                            start=(mc == 0),
                            stop=(mc == MC - 1),
                        )
                dst = locB if k % 2 == 1 else out1
                for j in range(NJ):
                    if k == 1:
                        nc.vector.tensor_copy(dst[j][:], ps_g[j][:])
                    else:
                        nc.vector.scalar_tensor_tensor(
                            dst[j][:], ps_g[j][:], 2.0, dst[j][:],
                            Alu.mult, Alu.subtract,
                        )
                conv2_feature(dst, k)
                if k < K - 1:
                    cc_in = drp.tile([R, C2], g_dt, tag="cc2i", name=f"cc2i_{k}")
                    for j in range(NJ):
                        cc_spill(cc_in[j * P:(j + 1) * P, :], dst[j][:])
                    cc_out = drp.tile(
                        [N, C2], g_dt, tag="cc2o", addr_space="Shared",
                        name=f"cc2o_{k}",
                    )
                    nc.gpsimd.collective_compute(
                        "AllGather", Alu.bypass, replica_groups=RG,
                        ins=[cc_in.opt()], outs=[cc_out.opt()],
                    )
                    src = cc_out

            # conv2 epilogue: out2 = relu(out2 + b2), cast for fc1
            fc_lhs = []
            for j in range(NJ):
                nc.vector.tensor_add(out2[j][:], out2[j][:], b2r[:])
                nc.vector.tensor_relu(out2[j][:], out2[j][:])
                t = out2p.tile([P, C2], fc1_dt, tag="out2c", name=f"out2c_{j}")
                nc.vector.tensor_copy(t[:], out2[j][:])
                fc_lhs.append(t)

            # =============================================================
            # fc1 (node-sharded contraction) + AllReduce
            # =============================================================
            fw1v = fw1s_d.rearrange("(j p f) m -> j f p m", p=P, f=F2)
            fc_ps = psp.tile([B, M1], f32, tag="ps", name="fc1_ps")
            n_mm = NJ * F2
            i_mm = 0
            for j in range(NJ):
                lhs_j = fc_lhs[j].rearrange("p (b f) -> p f b", f=F2)
                for f in range(F2):
                    fwt = fwp.tile([P, M1], fc1_dt, tag="fw", name=f"fw1_{j}_{f}")
                    nc.sync.dma_start(out=fwt[:], in_=fw1v[j, f])
                    nc.tensor.matmul(
                        fc_ps[:],
                        lhs_j[:, f, :],
                        fwt[:],
                        start=(i_mm == 0),
                        stop=(i_mm == n_mm - 1),
                    )
                    i_mm += 1

            z = fcp.tile([B, M1], f32)
            nc.vector.tensor_copy(z[:], fc_ps[:])
            cc_fi = drp.tile([B, M1], f32, tag="ccfi")
            nc.sync.dma_start(out=cc_fi[:], in_=z[:])
            cc_fo = drp.tile([B, M1], f32, tag="ccfo", addr_space="Shared")
            nc.gpsimd.collective_compute(
                "AllReduce", Alu.add, replica_groups=RG,
                ins=[cc_fi.opt()], outs=[cc_fo.opt()],
            )

            # z1 = relu(fc1 + fb1), padded to 32 partitions for PE transpose
            z1p = fcp.tile([32, M1], f32)
            nc.vector.memset(z1p[:], 0.0)
            nc.sync.dma_start(out=z1p[0:B, :], in_=cc_fo[:])
            nc.vector.tensor_add(z1p[0:B, :], z1p[0:B, :], fb1r[:])
            nc.vector.tensor_relu(z1p[0:B, :], z1p[0:B, :])

            # fc2
            fc2_ps = psp.tile([B, M2], f32, tag="ps", name="fc2_ps")
            for c in range(4):
                zt_ps = psp.tile([P, 32], f32, tag="ps", name=f"ztps_{c}")
                nc.tensor.transpose(
                    zt_ps[:], z1p[:, c * P:(c + 1) * P], ident[0:32, 0:32]
                )
                zt = fcp.tile([P, 32], f32r, tag="zt", name=f"zt_{c}")
                nc.vector.tensor_copy(zt[:], zt_ps[:])
                fwt2 = fcp.tile([P, M2], f32r, tag="fw2t", name=f"fw2t_{c}")
                nc.sync.dma_start(out=fwt2[:], in_=fw2_d[c * P:(c + 1) * P, :])
                nc.tensor.matmul(
                    fc2_ps[:],
                    zt[:, 0:B],
                    fwt2[:],
                    start=(c == 0),
                    stop=(c == 3),
                )
            z2p = fcp.tile([32, M2], f32)
            nc.vector.memset(z2p[:], 0.0)
            nc.vector.tensor_copy(z2p[0:B, :], fc2_ps[:])
            nc.vector.tensor_add(z2p[0:B, :], z2p[0:B, :], fb2r[:])
            nc.vector.tensor_relu(z2p[0:B, :], z2p[0:B, :])

            # fc3
            z3t_ps = psp.tile([P, 32], f32, tag="ps", name="z3tps")
            nc.tensor.transpose(z3t_ps[:], z2p[:], ident[0:32, 0:32])
            z3t = fcp.tile([P, 32], f32r)
            nc.vector.tensor_copy(z3t[:], z3t_ps[:])
            fc3_ps = psp.tile([B, M3], f32, tag="ps", name="fc3_ps")
            nc.tensor.matmul(
                fc3_ps[:], z3t[:, 0:B], fw3sb[:], start=True, stop=True
            )
            s = fcp.tile([B, M3], f32)
            nc.vector.tensor_copy(s[:], fc3_ps[:])
            nc.vector.tensor_add(s[:], s[:], fb3r[:])

            # softmax over the last dim (M3 = 2)
            mx = fcp.tile([B, 1], f32)
            nc.vector.reduce_max(mx[:], s[:], axis=mybir.AxisListType.X)
            nc.vector.tensor_scalar_mul(mx[:], mx[:], -1.0)
            nc.scalar.activation(s[:], s[:], Act.Exp, bias=mx[:, 0:1])
            sm = fcp.tile([B, 1], f32)
            nc.vector.reduce_sum(sm[:], s[:], axis=mybir.AxisListType.X)
            nc.vector.reciprocal(sm[:], sm[:])
            nc.vector.tensor_scalar_mul(s[:], s[:], sm[:, 0:1])
            nc.sync.dma_start(out=out_d[:], in_=s[:])

    nc.compile()
    return nc


def prepare_inputs(x, a, w1, b1, w2, b2, fw1, fb1, fw2, fb2, fw3, fb3):
    """Shard + re-layout the full model inputs into 8 per-core input maps."""
    import ml_dtypes

    g_np = np.float32 if DT_GRAPH == "f32" else ml_dtypes.bfloat16
    fc1_np = np.float32 if DT_FC1 == "f32" else ml_dtypes.bfloat16

    x = np.asarray(x, np.float32)
    a = np.asarray(a, np.float32)
    w1 = np.asarray(w1, np.float32)
    w2 = np.asarray(w2, np.float32)
    fw1 = np.asarray(fw1, np.float32)

    # node-major [N, B*F_IN] padded to C1
    x2d = x.transpose(1, 0, 2).reshape(N, B * F_IN).astype(np.float32)
    x2dp = np.zeros((N, C1), np.float32)
    x2dp[:, : B * F_IN] = x2d
    x2dp_c = x2dp.astype(g_np)

    w1bd = np.zeros((K, C1, C2), np.float32)
    for b in range(B):
        w1bd[:, b * F_IN:(b + 1) * F_IN, b * F1:(b + 1) * F1] = w1
    w2bd = np.zeros((K, P, P), np.float32)
    for q in range(4):
        w2bd[:, q * F1:(q + 1) * F1, q * F2:(q + 1) * F2] = w2

    b1r = np.broadcast_to(np.tile(np.asarray(b1, np.float32), B), (P, C2)).copy()
    b2r = np.broadcast_to(np.tile(np.asarray(b2, np.float32), B), (P, C2)).copy()
    fb1r = np.broadcast_to(np.asarray(fb1, np.float32), (B, M1)).copy()
    fb2r = np.broadcast_to(np.asarray(fb2, np.float32), (B, M2)).copy()
    fb3r = np.broadcast_to(np.asarray(fb3, np.float32), (B, M3)).copy()
    fw2_c = np.asarray(fw2, np.float32)
    fw3_c = np.asarray(fw3, np.float32)

    fw1_3 = fw1.reshape(N, F2, M1)

    in_maps = []
    for i in range(NCORES):
        r0 = i * R
        at_i = np.ascontiguousarray(a[r0:r0 + R, :].T).astype(g_np)
        xloc = (
            x2dp[r0:r0 + R]
            .reshape(NJ, P, C1)
            .transpose(1, 0, 2)
            .reshape(P, NJ * C1)
        )
        fw1s = np.ascontiguousarray(fw1_3[r0:r0 + R].reshape(R * F2, M1)).astype(
            fc1_np
        )
        in_maps.append(
            {
                "at": at_i,
                "x2dp": x2dp_c,
                "xloc": np.ascontiguousarray(xloc),
                "w1bd": w1bd,
                "w2bd": w2bd,
                "fw1s": fw1s,
                "fw2": fw2_c,
                "fw3": fw3_c,
                "b1r": b1r,
                "b2r": b2r,
                "fb1r": fb1r,
                "fb2r": fb2r,
                "fb3r": fb3r,
            }
        )
    return in_maps


def kernel(**inputs) -> np.ndarray:
    from concourse.bass_utils import run_bass_kernel_spmd

    key = (DT_GRAPH, DT_FC1)
    if key not in _CACHE:
        _CACHE[key] = build_kernel()
    nc = _CACHE[key]

    in_maps = prepare_inputs(**inputs)
    res = run_bass_kernel_spmd(nc, in_maps, core_ids=list(range(NCORES)))
    return np.asarray(res.results[0]["out"], np.float32)


if __name__ == "__main__":
    import importlib.util

    spec = importlib.util.spec_from_file_location(
        "reference", os.path.join(os.path.dirname(__file__), "reference.py")
    )
    ref = importlib.util.module_from_spec(spec)
    spec.loader.exec_module(ref)
    inputs = {k: np.asarray(v) for k, v in ref.setup_inputs().items()}
    out = kernel(**inputs)
    print(out)
